# revision 1
# baseline (speedup 1.0000x reference)
"""Causal self-attention TP kernel for 8 trn2 NeuronCores.

Problem shapes (hardcoded): x [2, 2048, 2048] f32, w_attn [2048, 6144],
w_proj [2048, 2048], 16 heads, head_dim 128.

Sharding: tensor-parallel over heads — core i owns heads {2i, 2i+1} for BOTH
batches. Each core computes its local-head qkv + attention, producing
y_local^T [512 feat, 4096 tok]. Two 8-core AllToAlls (one per batch)
re-shard from feature-split to token-split: core g receives
y^T[all 2048 feat, 256 tokens of each batch] and projects those 512 tokens
against the full w_proj, emitting out[512, 2048] (batch0 rows then batch1).
The batch-0 AllToAll overlaps batch-1 attention compute; the final
projection overlaps the batch-1 AllToAll.

All matmuls run as float32r (FP22, full PE rate at free-dim >= 256); the
BIR verifier requires fp32r matmul operands to be *produced* with fp32r
dtype, so operand tiles are allocated as F32R and producers write
f32r-labeled APs (identical 4-byte bits; the PE truncates on read).

Softmax: no max-subtraction (scores ~N(0,1), exp is safe in fp32); row
sums via a per-tile ones-matmul on the PE (partition-axis reduction);
normalization applied to y after the PV matmul via a DMA-broadcast
reciprocal.
"""

import os
import numpy as np

import concourse.bass as bass
import concourse.mybir as mybir
import concourse.tile as tile
from concourse import bacc
from concourse.bass_utils import run_bass_kernel_spmd

F32 = mybir.dt.float32
F32R = mybir.dt.float32r

B, T, C = 2, 2048, 2048
H, D = 16, 128
NTOK = B * T                     # 4096 flat tokens (batch-major)
SCALE = 1.0 / float(np.sqrt(D))  # 0.08838834764831845
NCORES = 8
HPC = H // NCORES                # 2 heads per core
FLOC = HPC * D                   # 256 local v features
QK = 512                         # q+k local features (2 heads x 128 x 2)

last_exec_time_ns = None
_cache = {}


def r32(ap):
    return ap.bitcast(F32R)


def _masks_np():
    # mask[m, kk, qq] = 1.0 iff kk <= qq - 128*m   (for diagonal tile offset m)
    m = np.arange(4)[:, None, None]
    kk = np.arange(128)[None, :, None]
    qq = np.arange(512)[None, None, :]
    return (kk <= qq - 128 * m).astype(np.float32)


def build_nc(no_collective=False, reps=1):
    nc = bacc.Bacc("TRN2", target_bir_lowering=False, debug=False,
                   num_devices=1 if no_collective else NCORES)

    xt = nc.dram_tensor("xt", [C, NTOK], F32, kind="ExternalInput")
    wqk = nc.dram_tensor("wqk", [C, QK], F32, kind="ExternalInput")
    wv = nc.dram_tensor("wv", [C, FLOC], F32, kind="ExternalInput")
    wp = nc.dram_tensor("wp", [C, C], F32, kind="ExternalInput")
    out = nc.dram_tensor("out", [512, C], F32, kind="ExternalOutput")

    v_dram = nc.dram_tensor("v_dram", [NTOK, FLOC], F32)
    # per-batch a2a buffers: 8 shards x [256 feat x 256 tok]
    y_loc = [nc.dram_tensor(f"y_loc{b}", [2048, 256], F32) for b in range(B)]
    y_t = [nc.dram_tensor(f"y_t{b}", [2048, 256], F32) for b in range(B)]
    masks = nc.inline_tensor(_masks_np(), "masks")      # [4, 128, 512]
    ones_dr = nc.inline_tensor(np.ones((128, 1), np.float32), "ones_c")
    zeros_dr = nc.inline_tensor(np.zeros((128, 1), np.float32), "zeros_c")
    onesr_dr = nc.inline_tensor(np.ones((1, 128), np.float32), "onesr_c")

    def a2a(b):
        if no_collective:
            nc.sync.dma_start(out=y_t[b][:, :], in_=y_loc[b][:, :])
        else:
            nc.gpsimd.collective_compute(
                "AllToAll",
                mybir.AluOpType.bypass,
                replica_groups=[list(range(NCORES))],
                ins=[y_loc[b][:, :]],
                outs=[y_t[b][:, :]],
            )

    with tile.TileContext(nc) as tc:
      for _rep in range(reps):
        # ---- persistent (qkv outputs + constants), freed after attention ----
        with tc.tile_pool(name="persist", bufs=1) as persist:
            # q^T,k^T for 2 heads, all tokens: chunk f = {q_h0, q_h1, k_h0, k_h1}
            qk_res = persist.tile([128, 4, NTOK], F32R)
            ones_sb = persist.tile([128, 1], F32R)
            zeros_sb = persist.tile([128, 1], F32)
            nc.gpsimd.dma_start(out=zeros_sb, in_=zeros_dr.ap())
            onesr_sb = persist.tile([1, 128], F32R)
            nc.gpsimd.dma_start(out=onesr_sb, in_=r32(onesr_dr.ap()))
            scr = persist.tile([128, 1], F32)
            # warm the ACT exp table set (~2.7us) before attention needs it
            nc.scalar.activation(scr, ones_sb.bitcast(F32),
                                 mybir.ActivationFunctionType.Exp, bias=zeros_sb)
            # constants go through the gpsimd DMA queue to keep the sync-engine
            # queue free for the latency-critical weight/x loads at startup
            nc.gpsimd.dma_start(out=ones_sb, in_=r32(ones_dr.ap()))

            # ================= phase 1: qkv =================
            p2v_ctx = tc.tile_pool(name="p2v", bufs=3)
            p2v = p2v_ctx.__enter__()
            v_pre = {}

            def load_v(b, h):
                v_sb = p2v.tile([128, 16, 128], F32R, tag="vsb",
                                name=f"v_sb{b}{h}")
                for vc in range(16):
                    nc.sync.dma_start(
                        out=v_sb[:, vc, :],
                        in_=r32(v_dram[b * T + vc * 128: b * T + (vc + 1) * 128,
                                       h * 128:(h + 1) * 128]),
                    )
                return v_sb

            with (
                tc.tile_pool(name="p1w", bufs=1) as p1w,
                tc.tile_pool(name="p1x", bufs=3) as p1x,
                tc.tile_pool(name="p1s", bufs=4) as p1s,
                tc.tile_pool(name="p1ps", bufs=4, space="PSUM") as p1ps,
                tc.tile_pool(name="p1psv", bufs=2, space="PSUM") as p1psv,
            ):
                wqk_sb = p1w.tile([128, 16, QK], F32R)
                wv_sb = p1w.tile([128, 16, FLOC], F32R)

                for tt in range(8):          # 512-token tiles over 4096 flat tokens
                    xh = []
                    for half in range(2):    # 8 c-chunks per half
                        xbuf = p1x.tile([128, 8, 512], F32R, tag="xh")
                        c0 = half * 8
                        for cc in range(8):
                            if tt == 0:
                                # interleave weight-chunk loads with the first
                                # x-tile loads so the c-loop matmuls can chase
                                # the DMA stream from ~2us in
                                nc.sync.dma_start(
                                    out=wqk_sb[:, c0 + cc, :],
                                    in_=r32(wqk[(c0 + cc) * 128:(c0 + cc + 1) * 128, :]))
                            nc.sync.dma_start(
                                out=xbuf[:, cc, :],
                                in_=r32(xt[(c0 + cc) * 128:(c0 + cc + 1) * 128,
                                           tt * 512:(tt + 1) * 512]),
                            )
                        xh.append(xbuf)
                    if tt == 0:
                        # wv is first needed ~14us in; load it after tt0's x
                        for c in range(16):
                            nc.sync.dma_start(
                                out=wv_sb[:, c, :],
                                in_=r32(wv[c * 128:(c + 1) * 128, :]))
                    # q^T / k^T feature blocks
                    for fb in range(4):
                        ps = p1ps.tile([128, 512], F32, tag="qkps")
                        for half in range(2):
                            for cc in range(8):
                                c = half * 8 + cc
                                nc.tensor.matmul(
                                    ps,
                                    lhsT=wqk_sb[:, c, fb * 128:(fb + 1) * 128],
                                    rhs=xh[half][:, cc, :],
                                    start=(c == 0), stop=(c == 15),
                                )
                        nc.vector.tensor_copy(qk_res[:, fb, tt * 512:(tt + 1) * 512], ps)
                    # v token blocks (token-major out)
                    for tb in range(4):
                        psv = p1psv.tile([128, FLOC], F32, tag="vps")
                        for half in range(2):
                            for cc in range(8):
                                c = half * 8 + cc
                                nc.tensor.matmul(
                                    psv,
                                    lhsT=xh[half][:, cc, tb * 128:(tb + 1) * 128],
                                    rhs=wv_sb[:, c, :],
                                    start=(c == 0), stop=(c == 15),
                                )
                        if tt < 4:
                            # batch-0 v goes straight to its attention tiles
                            if tt == 0 and tb == 0:
                                for h in range(HPC):
                                    v_pre[(0, h)] = p2v.tile(
                                        [128, 16, 128], F32R, tag="vsb",
                                        name=f"v_pre0{h}")
                            for h in range(HPC):
                                nc.vector.tensor_copy(
                                    v_pre[(0, h)][:, tt * 4 + tb, :],
                                    psv[:, h * 128:(h + 1) * 128])
                        else:
                            st = p1s.tile([128, FLOC], F32, tag="vst")
                            nc.vector.tensor_copy(st, psv)
                            nc.sync.dma_start(
                                out=v_dram[tt * 512 + tb * 128:
                                           tt * 512 + (tb + 1) * 128, :],
                                in_=st,
                            )

            # ============ phases 2+3: attention + per-batch a2a ============
            # proj pools open early so the first w_proj slice prefetches
            # during attention (their SBUF must not overlap phase-1 pools)
            with (
                tc.tile_pool(name="p4w", bufs=3) as p4w,
                tc.tile_pool(name="p4y", bufs=1) as p4y,
                tc.tile_pool(name="p4s", bufs=4) as p4s,
                tc.tile_pool(name="p4ps", bufs=1, space="PSUM") as p4ps,
            ):
                _wpn = [0]

                def load_wp_chunk(ch):
                    wt = p4w.tile([128, 16, 256], F32R, tag="wp",
                                  name=f"wp_t{_wpn[0]}")
                    _wpn[0] += 1
                    nc.sync.dma_start(
                        out=wt,
                        in_=r32(wp[:, ch * 256:(ch + 1) * 256].rearrange(
                            "(n p) f -> p n f", p=128)))
                    return wt

                def load_yt(b):
                    yb = p4y.tile([128, 16, 256], F32R, tag=f"yt{b}", name=f"yt{b}")
                    nc.sync.dma_start(
                        out=yb,
                        in_=r32(y_t[b].ap().rearrange("(n p) t -> p n t", p=128)))
                    return yb

                def proj_all(yts, wp_tiles):
                    """combined projection: one w_proj sweep. Batch-0 groups on
                    the prefetched chunks run first (they don't wait on the
                    batch-1 all-to-all); batch-1 + remaining chunks follow."""
                    npre = len(wp_tiles)
                    order = [(ch, 0) for ch in range(npre)]
                    order += [(ch, 1) for ch in range(npre)]
                    for ch in range(npre, 8):
                        order += [(ch, 0), (ch, 1)]
                    for ch, b in order:
                        if ch < npre:
                            wt = wp_tiles[ch]
                            # warm the stream: issue the next unloaded chunk's
                            # DMA one step ahead of its consuming groups
                            if b == 1 and ch + npre < 8 and len(wp_tiles) < 8:
                                wp_tiles.append(load_wp_chunk(len(wp_tiles)))
                        else:
                            while len(wp_tiles) <= min(ch + 1, 7):
                                wp_tiles.append(load_wp_chunk(len(wp_tiles)))
                            wt = wp_tiles[ch]
                        if True:
                            for tb in range(2):
                                ps = p4ps.tile([128, 256], F32, tag="ops")
                                for c in range(16):
                                    nc.tensor.matmul(
                                        ps,
                                        lhsT=yts[b][:, c, tb * 128:(tb + 1) * 128],
                                        rhs=wt[:, c, :],
                                        start=(c == 0), stop=(c == 15),
                                    )
                                st = p4s.tile([128, 256], F32, tag="ost")
                                nc.vector.tensor_copy(st, ps)
                                nc.sync.dma_start(
                                    out=out[b * 256 + tb * 128: b * 256 + (tb + 1) * 128,
                                            ch * 256:(ch + 1) * 256],
                                    in_=st,
                                )

                with (
                    tc.tile_pool(name="p2m", bufs=1) as p2m,
                    tc.tile_pool(name="p2p", bufs=5) as p2p,
                    tc.tile_pool(name="p2y", bufs=2) as p2y,
                    tc.tile_pool(name="p2r", bufs=3) as p2r,
                    tc.tile_pool(name="p2rd", bufs=2, space="DRAM") as p2rd,
                    tc.tile_pool(name="p2pss", bufs=3, space="PSUM") as p2pss,
                    tc.tile_pool(name="p2psy", bufs=2, space="PSUM") as p2psy,
                    tc.tile_pool(name="p2psr", bufs=2, space="PSUM") as p2psr,
                ):
                    mask_sb = p2m.tile([128, 4, 512], F32)
                    nc.sync.dma_start(out=mask_sb,
                                       in_=masks.ap().rearrange("m p q -> p m q"))
                    wp_pending = []   # w_proj chunks prefetched in b1 window
                    yts = []
                    for b in range(B):
                        nhj = 0
                        for h in range(HPC):
                            v_sb = v_pre.pop((b, h), None) or load_v(b, h)
                            qf, kf = h, 2 + h
                            tok0 = b * T
                            for j in range(4):
                                if b == 1 and nhj < 3:
                                    # stream next-proj w_proj chunk loads through
                                    # the sync FIFO during batch-1 attention
                                    wp_pending.append(load_wp_chunk(nhj))
                                    nhj += 1
                                nk = 4 * j + 4
                                y_ps = p2psy.tile([128, 512], F32, tag="yps")
                                r_ps = p2psr.tile([1, 512], F32, tag="rps")
                                qs = qk_res[:, qf, tok0 + j * 512: tok0 + (j + 1) * 512]
                                for c in range(nk):
                                    s_ps = p2pss.tile([128, 512], F32, tag="sps")
                                    nc.tensor.matmul(
                                        s_ps,
                                        lhsT=qk_res[:, kf,
                                                    tok0 + c * 128: tok0 + (c + 1) * 128],
                                        rhs=qs,
                                        start=True, stop=True,
                                    )
                                    p_sb = p2p.tile([128, 512], F32R, tag="p")
                                    nc.scalar.activation(
                                        p_sb, s_ps,
                                        mybir.ActivationFunctionType.Exp,
                                        scale=SCALE, bias=zeros_sb,
                                    )
                                    if c >= 4 * j:
                                        nc.vector.tensor_mul(
                                            p_sb, p_sb, mask_sb[:, c - 4 * j, :])
                                    nc.tensor.matmul(
                                        y_ps,
                                        lhsT=v_sb[:, c, :],
                                        rhs=p_sb,
                                        start=(c == 0), stop=(c == nk - 1),
                                    )
                                    nc.tensor.matmul(
                                        r_ps,
                                        lhsT=ones_sb,
                                        rhs=p_sb,
                                        start=(c == 0), stop=(c == nk - 1),
                                    )
                                rr = p2r.tile([1, 512], F32, tag="rr")
                                nc.vector.reciprocal(rr, r_ps)
                                rb = p2r.tile([128, 512], F32, tag="rb")
                                nc.gpsimd.partition_broadcast(rb, rr)
                                y_sb = p2y.tile([128, 512], F32, tag="ysb")
                                nc.vector.tensor_mul(y_sb, y_ps, rb)
                                # token eighths 2j, 2j+1 of batch b
                                for e in range(2):
                                    s = 2 * j + e
                                    nc.sync.dma_start(
                                        out=y_loc[b][s * 256 + h * 128:
                                                     s * 256 + (h + 1) * 128, :],
                                        in_=y_sb[:, e * 256:(e + 1) * 256],
                                    )
                        # batch-b all-to-all; b=0's overlaps b=1 attention
                        a2a(b)
                        yts.append(load_yt(b))
                    proj_all(yts, wp_pending)
            p2v_ctx.__exit__(None, None, None)

    nc.compile()
    return nc


def kernel(x, w_attn, w_proj):
    global last_exec_time_ns
    x = np.asarray(x, dtype=np.float32)
    w_attn = np.asarray(w_attn, dtype=np.float32)
    w_proj = np.asarray(w_proj, dtype=np.float32)

    if "nc" not in _cache:
        _cache["nc"] = build_nc()
    nc = _cache["nc"]

    xt = np.ascontiguousarray(x.reshape(NTOK, C).T)          # [C, NTOK]
    wp = np.ascontiguousarray(w_proj)
    in_maps = []
    for i in range(NCORES):
        qcols = w_attn[:, FLOC * i: FLOC * (i + 1)]
        kcols = w_attn[:, C + FLOC * i: C + FLOC * (i + 1)]
        vcols = w_attn[:, 2 * C + FLOC * i: 2 * C + FLOC * (i + 1)]
        in_maps.append({
            "xt": xt,
            "wqk": np.ascontiguousarray(np.concatenate([qcols, kcols], axis=1)),
            "wv": np.ascontiguousarray(vcols),
            "wp": wp,
        })

    res = run_bass_kernel_spmd(nc, in_maps, list(range(NCORES)))
    last_exec_time_ns = res.exec_time_ns

    return assemble([res.results[g]["out"] for g in range(NCORES)])


def assemble(outs):
    # core g's out rows: [0:256] = batch0 tokens [256g:256(g+1)],
    #                    [256:512] = batch1 tokens [256g:256(g+1)]
    full = np.empty((B, T, C), np.float32)
    for g in range(NCORES):
        for b in range(B):
            full[b, 256 * g: 256 * (g + 1), :] = outs[g][b * 256:(b + 1) * 256]
    return full



# revision 6
# speedup vs baseline: 1.2233x; 1.2233x over previous
"""Causal self-attention TP kernel for 8 trn2 NeuronCores.

Problem shapes (hardcoded): x [2, 2048, 2048] f32, w_attn [2048, 6144],
w_proj [2048, 2048], 16 heads, head_dim 128.

Sharding: tensor-parallel over heads - core i owns heads {2i, 2i+1} for BOTH
batches. Each core computes its local-head qkv + attention, producing
y_local^T [256 feat, 2048 tok] per batch. One 8-core AllToAll per batch
re-shards from feature-split to token-split: core g receives
y^T[all 2048 feat, 256 tokens of each batch] and projects those 512 tokens
against the full w_proj, emitting out[512, 2048] (batch0 rows then batch1).

Schedule (per core): P1a (batch-0 qkv) -> A0 (batch-0 attention) ->
a2a(0) overlapped with P1b (batch-1 qkv) -> A1 -> proj(b0) overlapped
with a2a(1) -> proj(b1). Both batches' V stays in SBUF (no DRAM
roundtrip). All activations/weights are bf16 (converted host-side);
PSUM accumulation is fp32, so precision loss is only input rounding
(~0.5% rms on the output, well inside the 2e-2 gate). bf16 halves DMA
traffic and the AllToAll payload (1MB/batch).

Queue discipline: SP (sync) queue carries only bulk loads (x, wqk, wv,
wp); y stores ride the DVE queue right behind the producing multiply;
collectives + y_t loads + constants ride the Pool queue so a parked
collective never blocks a latency-critical load.

Softmax: no max-subtraction (scores ~N(0,1), exp is safe); row sums via
a per-tile ones-matmul on the PE (partition-axis reduction), reciprocal
broadcast applied to y after the PV matmul. The attention inner loop is
software-pipelined one step (QK_{c+1} issued before PV_c/R_c) so the PE
never waits on the ACT exp latency.
"""

import numpy as np
import ml_dtypes

import concourse.bass as bass
import concourse.mybir as mybir
import concourse.tile as tile
from concourse import bacc
from concourse.bass_utils import run_bass_kernel_spmd

F32 = mybir.dt.float32
BF16 = mybir.dt.bfloat16
NPBF16 = ml_dtypes.bfloat16

B, T, C = 2, 2048, 2048
H, D = 16, 128
NTOK = B * T                     # 4096 flat tokens (batch-major)
SCALE = 1.0 / float(np.sqrt(D))
NCORES = 8
HPC = H // NCORES                # 2 heads per core
FLOC = HPC * D                   # 256 local v features
QK = 512                         # q+k local features (2 heads x 128 x 2)

last_exec_time_ns = None
_cache = {}


def _masks_np():
    # mask[m, kk, qq] = 1.0 iff kk <= qq - 128*m   (for diagonal tile offset m)
    m = np.arange(4)[:, None, None]
    kk = np.arange(128)[None, :, None]
    qq = np.arange(512)[None, None, :]
    return (kk <= qq - 128 * m).astype(NPBF16)


def build_nc(no_collective=False):
    nc = bacc.Bacc("TRN2", target_bir_lowering=False, debug=False,
                   num_devices=1 if no_collective else NCORES)

    xt = nc.dram_tensor("xt", [C, NTOK], BF16, kind="ExternalInput")
    wqk = nc.dram_tensor("wqk", [C, QK], BF16, kind="ExternalInput")
    wv = nc.dram_tensor("wv", [C, FLOC], BF16, kind="ExternalInput")
    wp = nc.dram_tensor("wp", [C, C], BF16, kind="ExternalInput")
    out = nc.dram_tensor("out", [512, C], F32, kind="ExternalOutput")

    # per-batch a2a buffers: 8 shards x [256 feat x 256 tok]
    y_loc = [nc.dram_tensor(f"y_loc{b}", [2048, 256], BF16) for b in range(B)]
    y_t = [nc.dram_tensor(f"y_t{b}", [2048, 256], BF16) for b in range(B)]
    masks = nc.inline_tensor(_masks_np(), "masks")      # [4, 128, 512] bf16
    ones_dr = nc.inline_tensor(np.ones((128, 1), NPBF16), "ones_c")
    zeros_dr = nc.inline_tensor(np.zeros((128, 1), np.float32), "zeros_c")

    def a2a(b):
        if no_collective:
            nc.gpsimd.dma_start(out=y_t[b][:, :], in_=y_loc[b][:, :])
        else:
            nc.gpsimd.collective_compute(
                "AllToAll",
                mybir.AluOpType.bypass,
                replica_groups=[list(range(NCORES))],
                ins=[y_loc[b][:, :]],
                outs=[y_t[b][:, :]],
            )

    with tile.TileContext(nc) as tc:
        # ---------------- persistent tiles ----------------
        with tc.tile_pool(name="persist", bufs=1) as persist:
            # q^T,k^T for 2 heads, all tokens: chunk f = {q_h0, q_h1, k_h0, k_h1}
            qk_res = persist.tile([128, 4, NTOK], BF16)
            # v, token-major, per (batch, head): [128 tok, 16 chunks, 128 feat]
            v_pre = {(b, h): persist.tile([128, 16, 128], BF16,
                                          name=f"v_pre{b}{h}")
                     for b in range(B) for h in range(HPC)}
            ones_sb = persist.tile([128, 1], BF16)
            zeros_sb = persist.tile([128, 1], F32)
            mask_sb = persist.tile([128, 4, 512], BF16)
            scr = persist.tile([128, 1], F32)
            # constants + masks go through the Pool (SWDGE) queue; SP queue is
            # reserved for the latency-critical weight/x loads at startup
            nc.gpsimd.dma_start(out=zeros_sb, in_=zeros_dr.ap())
            nc.gpsimd.dma_start(out=ones_sb, in_=ones_dr.ap())
            nc.gpsimd.dma_start(out=mask_sb,
                                in_=masks.ap().rearrange("m p q -> p m q"))
            # warm the ACT exp table set (~2.7us) before attention needs it
            nc.scalar.activation(scr, zeros_sb,
                                 mybir.ActivationFunctionType.Exp,
                                 bias=zeros_sb)

            # ---------------- pools ----------------
            # long-lived pools open first; phase-1 pools open innermost so
            # they can be released (LIFO) mid-kernel to free PSUM banks
            p2p_ctx = tc.tile_pool(name="p2p", bufs=5)
            p2y_ctx = tc.tile_pool(name="p2y", bufs=2)
            p2r_ctx = tc.tile_pool(name="p2r", bufs=2)
            p2pss_ctx = tc.tile_pool(name="p2pss", bufs=3, space="PSUM")
            p2psy_ctx = tc.tile_pool(name="p2psy", bufs=2, space="PSUM")
            p2psr_ctx = tc.tile_pool(name="p2psr", bufs=1, space="PSUM")
            p2p = p2p_ctx.__enter__()
            p2y = p2y_ctx.__enter__()
            p2r = p2r_ctx.__enter__()
            p2pss = p2pss_ctx.__enter__()
            p2psy = p2psy_ctx.__enter__()
            p2psr = p2psr_ctx.__enter__()
            p4w_ctx = tc.tile_pool(name="p4w", bufs=8)
            p4y_ctx = tc.tile_pool(name="p4y", bufs=1)
            p4s_ctx = tc.tile_pool(name="p4s", bufs=4)
            p4w = p4w_ctx.__enter__()
            p4y = p4y_ctx.__enter__()
            p4s = p4s_ctx.__enter__()

            p1w_ctx = tc.tile_pool(name="p1w", bufs=1)
            p1x_ctx = tc.tile_pool(name="p1x", bufs=3)
            p1ps_ctx = tc.tile_pool(name="p1ps", bufs=2, space="PSUM")
            p1w = p1w_ctx.__enter__()
            p1x = p1x_ctx.__enter__()
            p1ps = p1ps_ctx.__enter__()

            wqk_sb = p1w.tile([128, 16, QK], BF16)
            wv_sb = p1w.tile([128, 16, FLOC], BF16)

            def p1_half(bi):
                """qkv for batch bi's 2048 tokens (4 token tiles of 512)."""
                for ti in range(4):
                    tt = 4 * bi + ti
                    xh = []
                    for half in range(2):
                        xbuf = p1x.tile([128, 8, 512], BF16, tag="xh")
                        if tt == 0:
                            # first x tile + weights interleaved so the first
                            # matmul group can start as soon as possible
                            nc.sync.dma_start(
                                out=wqk_sb[:, half * 8:(half + 1) * 8, :],
                                in_=wqk[half * 1024:(half + 1) * 1024, :]
                                .rearrange("(n p) f -> p n f", p=128))
                        nc.sync.dma_start(
                            out=xbuf,
                            in_=xt[half * 1024:(half + 1) * 1024,
                                   tt * 512:(tt + 1) * 512]
                            .rearrange("(n p) f -> p n f", p=128))
                        xh.append(xbuf)
                    if tt == 0:
                        nc.sync.dma_start(
                            out=wv_sb,
                            in_=wv.ap().rearrange("(n p) f -> p n f", p=128))
                    # q^T / k^T feature blocks
                    for fb in range(4):
                        ps = p1ps.tile([128, 512], F32, tag="ps")
                        for c in range(16):
                            nc.tensor.matmul(
                                ps,
                                lhsT=wqk_sb[:, c, fb * 128:(fb + 1) * 128],
                                rhs=xh[c // 8][:, c % 8, :],
                                start=(c == 0), stop=(c == 15),
                            )
                        nc.vector.tensor_copy(
                            qk_res[:, fb, tt * 512:(tt + 1) * 512], ps)
                    # v token blocks (token-major out)
                    for tb in range(4):
                        psv = p1ps.tile([128, 512], F32, tag="ps")
                        for c in range(16):
                            nc.tensor.matmul(
                                psv[:, :FLOC],
                                lhsT=xh[c // 8][:, c % 8, tb * 128:(tb + 1) * 128],
                                rhs=wv_sb[:, c, :],
                                start=(c == 0), stop=(c == 15),
                            )
                        for h in range(HPC):
                            nc.vector.tensor_copy(
                                v_pre[(bi, h)][:, ti * 4 + tb, :],
                                psv[:, h * 128:(h + 1) * 128])

            def attn(b):
                tok0 = b * T
                for h in range(HPC):
                    v_sb = v_pre[(b, h)]
                    qf, kf = h, 2 + h
                    for j in range(4):
                        nk = 4 * j + 4
                        y_ps = p2psy.tile([128, 512], F32, tag="yps")
                        r_ps = p2psr.tile([1, 512], F32, tag="rps")
                        qs = qk_res[:, qf,
                                    tok0 + j * 512: tok0 + (j + 1) * 512]
                        pend = None
                        for c in range(nk):
                            s_ps = p2pss.tile([128, 512], F32, tag="sps")
                            nc.tensor.matmul(
                                s_ps,
                                lhsT=qk_res[:, kf,
                                            tok0 + c * 128: tok0 + (c + 1) * 128],
                                rhs=qs,
                                start=True, stop=True,
                            )
                            p_sb = p2p.tile([128, 512], BF16, tag="p")
                            nc.scalar.activation(
                                p_sb, s_ps,
                                mybir.ActivationFunctionType.Exp,
                                scale=SCALE, bias=zeros_sb,
                            )
                            if c >= 4 * j:
                                nc.vector.tensor_mul(
                                    p_sb, p_sb, mask_sb[:, c - 4 * j, :])
                            if pend is not None:
                                pc, pp = pend
                                nc.tensor.matmul(
                                    y_ps, lhsT=v_sb[:, pc, :], rhs=pp,
                                    start=(pc == 0), stop=False)
                                nc.tensor.matmul(
                                    r_ps, lhsT=ones_sb, rhs=pp,
                                    start=(pc == 0), stop=False)
                            pend = (c, p_sb)
                        pc, pp = pend
                        nc.tensor.matmul(y_ps, lhsT=v_sb[:, pc, :], rhs=pp,
                                         start=(pc == 0), stop=True)
                        nc.tensor.matmul(r_ps, lhsT=ones_sb, rhs=pp,
                                         start=(pc == 0), stop=True)
                        rr = p2r.tile([1, 512], F32, tag="rr")
                        nc.vector.reciprocal(rr, r_ps)
                        rb = p2r.tile([128, 512], F32, tag="rb")
                        nc.gpsimd.partition_broadcast(rb, rr)
                        y_sb = p2y.tile([128, 512], BF16, tag="ysb")
                        nc.vector.tensor_mul(y_sb, y_ps, rb)
                        # token eighths 2j, 2j+1 of batch b; stores ride the
                        # DVE queue right behind the producing multiply
                        for e in range(2):
                            s = 2 * j + e
                            nc.gpsimd.dma_start(
                                out=y_loc[b][s * 256 + h * 128:
                                             s * 256 + (h + 1) * 128, :],
                                in_=y_sb[:, e * 256:(e + 1) * 256],
                            )

            def load_yt(b):
                yb = p4y.tile([128, 16, 256], BF16, tag=f"yt{b}",
                              name=f"yt{b}")
                nc.gpsimd.dma_start(
                    out=yb,
                    in_=y_t[b].ap().rearrange("(n p) t -> p n t", p=128))
                return yb

            def proj(b, yb, wp_tiles, ps_pool):
                for ch in range(8):
                    wt = wp_tiles[ch]
                    for tb in range(2):
                        ps = ps_pool.tile([128, 256], F32, tag="ops")
                        for c in range(16):
                            nc.tensor.matmul(
                                ps,
                                lhsT=yb[:, c, tb * 128:(tb + 1) * 128],
                                rhs=wt[:, c, :],
                                start=(c == 0), stop=(c == 15),
                            )
                        st = p4s.tile([128, 256], F32, tag="ost")
                        nc.vector.tensor_copy(st, ps)
                        nc.scalar.dma_start(
                            out=out[b * 256 + tb * 128:
                                    b * 256 + (tb + 1) * 128,
                                    ch * 256:(ch + 1) * 256],
                            in_=st,
                        )

            # ================= schedule =================
            p1_half(0)
            attn(0)
            a2a(0)
            yt0 = load_yt(0)          # Pool queue, parked behind a2a(0)
            p1_half(1)
            # w_proj chunks stream on the SP queue behind batch-1's x loads
            wp_tiles = []
            for ch in range(8):
                wt = p4w.tile([128, 16, 256], BF16, tag="wp",
                              name=f"wp_t{ch}")
                nc.sync.dma_start(
                    out=wt,
                    in_=wp[:, ch * 256:(ch + 1) * 256]
                    .rearrange("(n p) f -> p n f", p=128))
                wp_tiles.append(wt)
            # phase-1 pools done (LIFO); free their PSUM banks for the proj
            p1ps_ctx.__exit__(None, None, None)
            p1x_ctx.__exit__(None, None, None)
            p1w_ctx.__exit__(None, None, None)
            p4ps_ctx = tc.tile_pool(name="p4ps", bufs=2, space="PSUM")
            p4ps = p4ps_ctx.__enter__()

            attn(1)
            a2a(1)
            yt1 = load_yt(1)
            proj(0, yt0, wp_tiles, p4ps)   # overlaps a2a(1) flight
            proj(1, yt1, wp_tiles, p4ps)

            for ctx in (p4ps_ctx, p4s_ctx, p4y_ctx, p4w_ctx, p2psr_ctx,
                        p2psy_ctx, p2pss_ctx, p2r_ctx, p2y_ctx, p2p_ctx):
                ctx.__exit__(None, None, None)

    nc.compile()
    return nc


def _in_maps(x, w_attn, w_proj):
    xt = np.ascontiguousarray(
        x.reshape(NTOK, C).T).astype(NPBF16)              # [C, NTOK]
    wp = np.ascontiguousarray(w_proj).astype(NPBF16)
    maps = []
    for i in range(NCORES):
        qcols = w_attn[:, FLOC * i: FLOC * (i + 1)]
        kcols = w_attn[:, C + FLOC * i: C + FLOC * (i + 1)]
        vcols = w_attn[:, 2 * C + FLOC * i: 2 * C + FLOC * (i + 1)]
        maps.append({
            "xt": xt,
            "wqk": np.ascontiguousarray(
                np.concatenate([qcols, kcols], axis=1)).astype(NPBF16),
            "wv": np.ascontiguousarray(vcols).astype(NPBF16),
            "wp": wp,
        })
    return maps


def kernel(x, w_attn, w_proj):
    global last_exec_time_ns
    x = np.asarray(x, dtype=np.float32)
    w_attn = np.asarray(w_attn, dtype=np.float32)
    w_proj = np.asarray(w_proj, dtype=np.float32)

    if "nc" not in _cache:
        _cache["nc"] = build_nc()
    nc = _cache["nc"]

    res = run_bass_kernel_spmd(nc, _in_maps(x, w_attn, w_proj),
                               list(range(NCORES)))
    last_exec_time_ns = res.exec_time_ns

    return assemble([res.results[g]["out"] for g in range(NCORES)])


def assemble(outs):
    # core g's out rows: [0:256] = batch0 tokens [256g:256(g+1)],
    #                    [256:512] = batch1 tokens [256g:256(g+1)]
    full = np.empty((B, T, C), np.float32)
    for g in range(NCORES):
        for b in range(B):
            full[b, 256 * g: 256 * (g + 1), :] = outs[g][b * 256:(b + 1) * 256]
    return full
